# revision 1
# baseline (speedup 1.0000x reference)
"""GCN2 (GCNII) on 8 Trainium2 NeuronCores.

Strategy: nodes degree-sorted and round-robin sharded across cores (targets).
Edge weights ew = dinv[src]*dinv[tgt] are separable: the table holds
h~ = dinv*h, aggregation is an unweighted segment-sum done entirely by
CCE-accumulating [128,1] indirect-DMA gathers into per-block SBUF
accumulators (column-ELL over degree-sorted targets). Epilogue per layer:
z = 0.9*dinv*acc + 0.1*x0, z2 = z @ ((1-b)I + bW) via PE transpose +
matmul, BatchNorm stats via ones-matmul + AllReduce, apply+relu on ACT in
feature-major, dinv-scale, transpose back, AllGather the new table.
"""
import numpy as np

import bass_rust
from concourse import bass, mybir
from concourse.bass_utils import run_bass_kernel_spmd
from concourse import tile as _tile
from concourse.vector_clock import ScopedClock

# ---------------------------------------------------------------- tctx patch
MAX_WAITS_PER_INST = 1


def _split_sync_waits(nc, max_waits=MAX_WAITS_PER_INST):
    for bb in nc.main_func.blocks:
        out = []
        changed = False
        for ins in bb.instructions:
            si = ins.sync_info
            waits = list(si.on_wait) if si is not None else []
            if len(waits) > max_waits:
                changed = True
                extra, keep = waits[:-max_waits], waits[-max_waits:]
                for i in range(0, len(extra), max_waits):
                    nop = bass_rust.InstNoOp(
                        name=nc.get_next_instruction_name(), text_hint="wsplit"
                    )
                    nop.engine = ins.engine
                    nop.sync_info = mybir.SyncInfo(
                        on_wait=extra[i : i + max_waits], on_update=[]
                    )
                    nc.register_instruction(nop, overwrite=True)
                    out.append(nop)
                ins.sync_info = mybir.SyncInfo(on_wait=keep, on_update=list(si.on_update))
            out.append(ins)
        if changed:
            bb.instructions = out


class TC(_tile.TileContext):
    def __exit__(self, *args):
        r = super().__exit__(*args)
        _split_sync_waits(self.nc)
        return r


# ---------------------------------------------------------------- config
FULL = dict(N=50000, E=800000, IN=500, HID=96, OUT=40, LAYERS=8)
P = 128
CORES = 8
ALPHA, THETA, BN_EPS = 0.1, 0.5, 1e-5
F32 = mybir.dt.float32


def host_prep(x, edge_index, W0, b0, Ws, bn_gamma, bn_beta, W_out, b_out, cfg):
    N, E, IN, HID, OUT, L = (cfg[k] for k in ("N", "E", "IN", "HID", "OUT", "LAYERS"))
    SLOTS = ((N // CORES) + P - 1) // P * P  # per-core slots (128-mult)
    B = SLOTS // P
    row, col = np.asarray(edge_index[0]), np.asarray(edge_index[1])
    deg = np.bincount(col, minlength=N).astype(np.float32) + 1.0  # self-loop
    dinv = 1.0 / np.sqrt(deg)

    order = np.argsort(-deg, kind="stable")  # rank -> node
    rank = np.empty(N, dtype=np.int64)
    rank[order] = np.arange(N)
    # rank r -> core r%8, slot r//8 ; table row = core*SLOTS + slot
    node_core = rank % CORES
    node_slot = rank // CORES
    node_row = node_core * SLOTS + node_slot
    ZERO_ROW = (CORES - 1) * SLOTS + (SLOTS - 1)  # dummy slot, h~ == 0 always

    # per-core ELL of incoming edges (self first), slots are degree-sorted
    tgt_core = node_core[col]
    tgt_slot = node_slot[col]
    src_row_all = node_row[row]
    d_real = np.zeros((CORES, SLOTS), dtype=np.int64)  # includes self
    ells = []
    for c in range(CORES):
        m = tgt_core == c
        s = tgt_slot[m]
        sr = src_row_all[m]
        cnt = np.bincount(s, minlength=SLOTS)
        nreal = min(SLOTS, (N - c + CORES - 1) // CORES)
        d = cnt + (np.arange(SLOTS) < nreal).astype(np.int64)  # +self
        d_real[c] = d
        dmax = int(d.max())
        ell = np.full((SLOTS, dmax), ZERO_ROW, dtype=np.int32)
        # self edge in column 0 for real slots
        sl = np.arange(nreal)
        ell[sl, 0] = (c * SLOTS + sl).astype(np.int32)
        # remaining edges in columns 1.., grouped by slot
        o = np.argsort(s, kind="stable")
        s_s, sr_s = s[o], sr[o]
        pos = np.arange(len(s_s)) - np.searchsorted(s_s, s_s) + 1
        ell[s_s, pos] = sr_s.astype(np.int32)
        ells.append(ell)

    dmax_g = max(e.shape[1] for e in ells)
    # global column sizes (max over cores), padded to 128
    C = [SLOTS]  # column 0 initializes every block
    for k in range(1, dmax_g):
        ck = max(int((d_real[c] > k).sum()) for c in range(CORES))
        ck = (ck + P - 1) // P * P
        if ck == 0:
            break
        C.append(ck)
    calls = []  # (k, blk, bypass)
    for k, ck in enumerate(C):
        for blk in range(ck // P):
            calls.append((k, blk, k == 0))
    NCALLS = len(calls)

    idx_maps, x_shards, d9s, d10s, drows, d1s, masks = [], [], [], [], [], [], []
    for c in range(CORES):
        ell = ells[c]
        idx = np.full((P, NCALLS), ZERO_ROW, dtype=np.int32)
        for ci, (k, blk, _) in enumerate(calls):
            if k < ell.shape[1]:
                sl = np.arange(blk * P, blk * P + P)
                idx[:, ci] = ell[sl, k]
        idx_maps.append(idx)
        r = np.arange(SLOTS) * CORES + c
        valid = r < N
        nd = np.where(valid, order[np.minimum(r, N - 1)], 0)
        xs = np.zeros((SLOTS, IN), dtype=np.float32)
        xs[valid] = np.asarray(x)[nd[valid]]
        x_shards.append(xs)
        dv = np.where(valid, dinv[nd], 0.0).astype(np.float32)
        d9s.append((0.9 * dv).reshape(B, P).T.copy())   # [P, B]
        d1s.append(dv.reshape(B, P).T.copy())
        masks.append(valid.astype(np.float32).reshape(B, P).T.copy())
        d10s.append((10.0 * dv).reshape(B, P).T.copy())
        drows.append(dv.reshape(1, SLOTS).copy())

    Ms = np.zeros((L * HID, HID), dtype=np.float32)
    for i in range(1, L + 1):
        bt = float(np.log(THETA / i + 1.0))
        Ms[(i - 1) * HID : i * HID] = (1 - bt) * np.eye(HID, dtype=np.float32) + bt * np.asarray(Ws)[i - 1]
    bnT = np.concatenate([np.asarray(bn_gamma).T, np.asarray(bn_beta).T], axis=1).astype(np.float32)  # [HID, 2*(L+1)]
    meta = dict(SLOTS=SLOTS, B=B, NCALLS=NCALLS, calls=calls, order=order)
    shared = dict(
        W0=np.asarray(W0, np.float32), Ms=Ms, bnT=bnT,
        W_out=np.asarray(W_out, np.float32),
        b_out=np.asarray(b_out, np.float32).reshape(OUT, 1),
        ident=np.eye(P, dtype=np.float32),
    )
    in_maps = []
    for c in range(CORES):
        m = dict(shared)
        m.update(x=x_shards[c], idx=idx_maps[c], d9=d9s[c], d10=d10s[c], drow=drows[c], d1=d1s[c], mask=masks[c])
        in_maps.append(m)
    return in_maps, meta


def build(cfg, meta):
    N, IN, HID, OUT, L = (cfg[k] for k in ("N", "IN", "HID", "OUT", "LAYERS"))
    SLOTS, B, NCALLS, calls = meta["SLOTS"], meta["B"], meta["NCALLS"], meta["calls"]
    TOT = CORES * SLOTS
    KC = (IN + P - 1) // P  # k-chunks for input matmul
    ksz = [min(P, IN - i * P) for i in range(KC)]

    nc = bass.Bass()
    x_in = nc.declare_dram_parameter("x", [SLOTS, IN], F32, isOutput=False)
    idx_in = nc.declare_dram_parameter("idx", [P, NCALLS], mybir.dt.int32, isOutput=False)
    d9_in = nc.declare_dram_parameter("d9", [P, B], F32, isOutput=False)
    d10_in = nc.declare_dram_parameter("d10", [P, B], F32, isOutput=False)
    drow_in = nc.declare_dram_parameter("drow", [1, SLOTS], F32, isOutput=False)
    d1_in = nc.declare_dram_parameter("d1", [P, B], F32, isOutput=False)
    mask_in = nc.declare_dram_parameter("mask", [P, B], F32, isOutput=False)
    W0_in = nc.declare_dram_parameter("W0", [IN, HID], F32, isOutput=False)
    Ms_in = nc.declare_dram_parameter("Ms", [L * HID, HID], F32, isOutput=False)
    bnT_in = nc.declare_dram_parameter("bnT", [HID, 2 * (L + 1)], F32, isOutput=False)
    Wout_in = nc.declare_dram_parameter("W_out", [HID, OUT], F32, isOutput=False)
    bout_in = nc.declare_dram_parameter("b_out", [OUT, 1], F32, isOutput=False)
    id_in = nc.declare_dram_parameter("ident", [P, P], F32, isOutput=False)
    out_ext = nc.declare_dram_parameter("out", [SLOTS, OUT], F32, isOutput=True)

    tables = [nc.dram_tensor(f"table{i}", [TOT, HID], F32) for i in range(L)]
    shards = [nc.dram_tensor(f"shard{i}", [SLOTS, HID], F32) for i in range(L)]
    st_in = [nc.dram_tensor(f"stin{i}", [2 * HID, 1], F32) for i in range(L + 1)]
    st_out = [nc.dram_tensor(f"stout{i}", [2 * HID, 1], F32, addr_space="Shared") for i in range(L + 1)]
    bn0_scr = nc.dram_tensor("bn0scr", [2 * HID, 1], F32)
    RG = [list(range(CORES))]

    with TC(nc, num_cores=CORES) as tc:
        with (
            tc.tile_pool(name="persist", bufs=1) as pp,
            tc.tile_pool(name="work", bufs=3) as wp,
            tc.tile_pool(name="acc", bufs=2) as ap,
            tc.tile_pool(name="psum", bufs=1, space="PSUM") as psp,
        ):
            idx_t = pp.tile([P, NCALLS], mybir.dt.int32, tag="idx")
            nc.sync.dma_start(out=idx_t[:], in_=idx_in[:])
            d9_t = pp.tile([P, B], F32, tag="d9")
            nc.sync.dma_start(out=d9_t[:], in_=d9_in[:])
            d10_t = pp.tile([P, B], F32, tag="d10")
            nc.sync.dma_start(out=d10_t[:], in_=d10_in[:])
            d1_t = pp.tile([P, B], F32, tag="d1")
            nc.sync.dma_start(out=d1_t[:], in_=d1_in[:])
            mask_t = pp.tile([P, B], F32, tag="mask")
            nc.sync.dma_start(out=mask_t[:], in_=mask_in[:])
            ident = pp.tile([P, P], F32, tag="id")
            nc.sync.dma_start(out=ident[:], in_=id_in[:])
            W0c = []
            for kc in range(KC):
                kw = ksz[kc]
                w = pp.tile([P, HID], F32, tag=f"w0_{kc}")
                nc.sync.dma_start(out=w[:kw, :], in_=W0_in[kc * P : kc * P + kw, :])
                W0c.append(w)
            Wo_t = pp.tile([HID, OUT], F32, tag="wo")
            nc.sync.dma_start(out=Wo_t[:], in_=Wout_in[:])
            bo_t = pp.tile([OUT, 1], F32, tag="bo")
            nc.sync.dma_start(out=bo_t[:], in_=bout_in[:])
            bnT_t = pp.tile([HID, 2 * (L + 1)], F32, tag="bn")
            nc.sync.dma_start(out=bnT_t[:], in_=bnT_in[:])
            ones = pp.tile([P, 1], F32, tag="ones")
            nc.vector.memset(ones[:], 1.0)

            x0s = pp.tile([P, B, HID], F32, tag="x0s")       # 0.1 * x0
            stage = pp.tile([P, B, HID], F32, tag="stage")   # h~ node-major
            z1st = pp.tile([P, B, HID], F32, tag="z1st")
            stc1 = pp.tile([HID, B], F32, tag="stc1")
            stc2 = pp.tile([HID, B], F32, tag="stc2")

            nreal = min(SLOTS, (N + CORES - 1) // CORES)
            dp0, db0 = nreal % P, nreal // P  # first dummy (p, b)

            def bn_vectors(sum_ps, sum2_ps, gcol, bcol, sc, bi):
                """scale/bias [HID,1] from per-partition sums + gamma/beta cols."""
                m = wp.tile([HID, 1], F32, tag="bnm")
                nc.vector.tensor_scalar_mul(m[:], sum_ps, 1.0 / N)
                v = wp.tile([HID, 1], F32, tag="bnv")
                nc.vector.tensor_scalar_mul(v[:], sum2_ps, 1.0 / N)
                m2 = wp.tile([HID, 1], F32, tag="bnm2")
                nc.vector.tensor_tensor(out=m2[:], in0=m[:], in1=m[:], op=mybir.AluOpType.mult)
                nc.vector.tensor_tensor(out=v[:], in0=v[:], in1=m2[:], op=mybir.AluOpType.subtract)
                nc.vector.tensor_scalar_add(v[:], v[:], BN_EPS)
                sd = wp.tile([HID, 1], F32, tag="bnsd")
                nc.scalar.activation(out=sd[:], in_=v[:], func=mybir.ActivationFunctionType.Sqrt)
                nc.vector.reciprocal(out=sd[:], in_=sd[:])
                nc.vector.tensor_tensor(out=sc[:], in0=bnT_t[:, gcol : gcol + 1], in1=sd[:], op=mybir.AluOpType.mult)
                t = wp.tile([HID, 1], F32, tag="bnt")
                nc.vector.tensor_tensor(out=t[:], in0=m[:], in1=sc[:], op=mybir.AluOpType.mult)
                nc.vector.tensor_tensor(out=bi[:], in0=bnT_t[:, bcol : bcol + 1], in1=t[:], op=mybir.AluOpType.subtract)

            # ---------------- layer 0: z1 = x @ W0 (+b0==0), BN0, relu
            s1_ps = psp.tile([HID, 1], F32, tag="s1")
            s2_ps = psp.tile([HID, 1], F32, tag="s2")
            for b in range(B):
                xb = wp.tile([P, IN], F32, tag="xb")
                nc.sync.dma_start(out=xb[:], in_=x_in[b * P : (b + 1) * P, :])
                z1_ps = psp.tile([P, HID], F32, tag="z1ps")
                for kc in range(KC):
                    kw = ksz[kc]
                    xt_ps = psp.tile([P, P], F32, tag="mm", bufs=4)
                    nc.tensor.transpose(out=xt_ps[:kw, :], in_=xb[:, kc * P : kc * P + kw], identity=ident[:])
                    xt_sb = wp.tile([P, P], F32, tag="xtsb")
                    nc.vector.tensor_copy(out=xt_sb[:kw, :], in_=xt_ps[:kw, :])
                    nc.tensor.matmul(
                        out=z1_ps[:], lhsT=xt_sb[:kw, :], rhs=W0c[kc][:kw, :],
                        start=(kc == 0), stop=(kc == KC - 1),
                    )
                nc.scalar.activation(out=z1st[:, b, :], in_=z1_ps[:], func=mybir.ActivationFunctionType.Copy)
                nc.tensor.matmul(out=s1_ps[:], lhsT=z1st[:, b, :], rhs=ones[:], start=(b == 0), stop=(b == B - 1))
                sq = wp.tile([P, HID], F32, tag="sq")
                nc.scalar.activation(out=sq[:], in_=z1st[:, b, :], func=mybir.ActivationFunctionType.Square)
                nc.tensor.matmul(out=s2_ps[:], lhsT=sq[:], rhs=ones[:], start=(b == 0), stop=(b == B - 1))
            sum1 = wp.tile([HID, 1], F32, tag="sum1")
            nc.vector.tensor_copy(out=sum1[:], in_=s1_ps[:])
            sum2 = wp.tile([HID, 1], F32, tag="sum2")
            nc.vector.tensor_copy(out=sum2[:], in_=s2_ps[:])
            nc.sync.dma_start(out=st_in[0][0:HID, :], in_=sum1[:])
            nc.sync.dma_start(out=st_in[0][HID : 2 * HID, :], in_=sum2[:])
            nc.gpsimd.collective_compute(
                "AllReduce", mybir.AluOpType.add, replica_groups=RG,
                ins=[st_in[0][:]], outs=[st_out[0][:]],
            )
            asum1 = wp.tile([HID, 1], F32, tag="as1")
            nc.sync.dma_start(out=asum1[:], in_=st_out[0][0:HID, :])
            asum2 = wp.tile([HID, 1], F32, tag="as2")
            nc.sync.dma_start(out=asum2[:], in_=st_out[0][HID : 2 * HID, :])
            sc0 = pp.tile([HID, 1], F32, tag="sc0")
            bi0 = pp.tile([HID, 1], F32, tag="bi0")
            bn_vectors(asum1[:], asum2[:], 0, L + 1, sc0, bi0)
            nc.vector.tensor_scalar_mul(sc0[:], sc0[:], 0.1)
            nc.vector.tensor_scalar_mul(bi0[:], bi0[:], 0.1)
            for b in range(B):
                z1T_ps = psp.tile([HID, P], F32, tag="mm", bufs=4)
                nc.tensor.transpose(out=z1T_ps[:], in_=z1st[:, b, :], identity=ident[:])
                x0T = wp.tile([HID, P], F32, tag="x0T")
                nc.scalar.activation(out=x0T[:], in_=z1T_ps[:], func=mybir.ActivationFunctionType.Relu, scale=sc0[:], bias=bi0[:])
                x0b_ps = psp.tile([P, HID], F32, tag="mm", bufs=4)
                nc.tensor.transpose(out=x0b_ps[:], in_=x0T[:], identity=ident[:HID, :HID])
                nc.vector.tensor_scalar(out=x0s[:, b, :], in0=x0b_ps[:], scalar1=mask_t[:, b : b + 1], scalar2=None, op0=mybir.AluOpType.mult)
            for b in range(B):
                nc.vector.tensor_scalar(out=stage[:, b : b + 1, :], in0=x0s[:, b : b + 1, :], scalar1=d10_t[:, b : b + 1], scalar2=None, op0=mybir.AluOpType.mult)
            nc.sync.dma_start(out=shards[0][:].rearrange("(b p) d -> p b d", p=P), in_=stage[:])
            nc.gpsimd.collective_compute(
                "AllGather", mybir.AluOpType.bypass, replica_groups=RG,
                ins=[shards[0][:]], outs=[tables[0][:]],
            )

            # ---------------- layers 1..L
            z2st = pp.tile([HID, B, P], F32, tag="z2st")
            for li in range(1, L + 1):
                tbl = tables[li - 1]
                Ms_t_cur = wp.tile([HID, HID], F32, tag="ms")
                nc.sync.dma_start(out=Ms_t_cur[:], in_=Ms_in[(li - 1) * HID : li * HID, :])
                accs = [ap.tile([P, HID], F32, tag=f"acc{b}", name=f"acc{li}_{b}") for b in range(B)]
                for ci, (k, blk, bypass) in enumerate(calls):
                    nc.gpsimd.indirect_dma_start(
                        out=accs[blk][:], out_offset=None, in_=tbl[:],
                        in_offset=bass.IndirectOffsetOnAxis(ap=idx_t[:, ci : ci + 1], axis=0),
                        compute_op=mybir.AluOpType.bypass if bypass else mybir.AluOpType.add,
                    )
                skip_bn = li == L - 1
                for b in range(B):
                    zb = wp.tile([P, HID], F32, tag="zb")
                    nc.vector.scalar_tensor_tensor(
                        out=zb[:], in0=accs[b][:], scalar=d9_t[:, b : b + 1],
                        in1=x0s[:, b, :], op0=mybir.AluOpType.mult, op1=mybir.AluOpType.add,
                    )
                    zT_ps = psp.tile([HID, P], F32, tag="mm", bufs=4)
                    nc.tensor.transpose(out=zT_ps[:], in_=zb[:], identity=ident[:])
                    zT = wp.tile([HID, P], F32, tag="zT")
                    nc.vector.tensor_copy(out=zT[:], in_=zT_ps[:])
                    z2_ps = psp.tile([HID, P], F32, tag="mm", bufs=4)
                    nc.tensor.matmul(out=z2_ps[:], lhsT=Ms_t_cur[:], rhs=zT[:], start=True, stop=True)
                    if skip_bn:
                        nc.vector.tensor_copy(out=z2st[:, b, :], in_=z2_ps[:])
                    else:
                        nc.scalar.activation(out=z2st[:, b, :], in_=z2_ps[:], func=mybir.ActivationFunctionType.Copy, accum_out=stc1[:, b : b + 1])
                        sq2 = wp.tile([HID, P], F32, tag="sq2")
                        nc.scalar.activation(out=sq2[:], in_=z2st[:, b, :], func=mybir.ActivationFunctionType.Square, accum_out=stc2[:, b : b + 1])
                if not skip_bn:
                    rs1 = wp.tile([HID, 1], F32, tag="rs1")
                    nc.vector.tensor_reduce(out=rs1[:], in_=stc1[:], axis=mybir.AxisListType.X, op=mybir.AluOpType.add)
                    rs2 = wp.tile([HID, 1], F32, tag="rs2")
                    nc.vector.tensor_reduce(out=rs2[:], in_=stc2[:], axis=mybir.AxisListType.X, op=mybir.AluOpType.add)
                    nc.sync.dma_start(out=st_in[li][0:HID, :], in_=rs1[:])
                    nc.sync.dma_start(out=st_in[li][HID : 2 * HID, :], in_=rs2[:])
                    nc.gpsimd.collective_compute(
                        "AllReduce", mybir.AluOpType.add, replica_groups=RG,
                        ins=[st_in[li][:]], outs=[st_out[li][:]],
                    )
                    g1 = wp.tile([HID, 1], F32, tag="g1")
                    nc.sync.dma_start(out=g1[:], in_=st_out[li][0:HID, :])
                    g2 = wp.tile([HID, 1], F32, tag="g2")
                    nc.sync.dma_start(out=g2[:], in_=st_out[li][HID : 2 * HID, :])
                    sc = wp.tile([HID, 1], F32, tag="sc")
                    bi = wp.tile([HID, 1], F32, tag="bi")
                    bn_vectors(g1[:], g2[:], li, (L + 1) + li, sc, bi)
                for b in range(B):
                    hT = wp.tile([HID, P], F32, tag="hT")
                    if skip_bn:
                        nc.scalar.activation(out=hT[:], in_=z2st[:, b, :], func=mybir.ActivationFunctionType.Relu)
                    else:
                        nc.scalar.activation(out=hT[:], in_=z2st[:, b, :], func=mybir.ActivationFunctionType.Relu, scale=sc[:], bias=bi[:])
                    if li < L:
                        hb_ps = psp.tile([P, HID], F32, tag="mm", bufs=4)
                        nc.tensor.transpose(out=hb_ps[:], in_=hT[:], identity=ident[:HID, :HID])
                        nc.vector.tensor_scalar(out=stage[:, b, :], in0=hb_ps[:], scalar1=d1_t[:, b : b + 1], scalar2=None, op0=mybir.AluOpType.mult)
                    else:
                        o_ps = psp.tile([OUT, P], F32, tag="mm", bufs=4)
                        nc.tensor.matmul(out=o_ps[:], lhsT=Wo_t[:], rhs=hT[:], start=True, stop=True)
                        oT = wp.tile([OUT, P], F32, tag="oT")
                        nc.vector.tensor_scalar(out=oT[:], in0=o_ps[:], scalar1=bo_t[:], scalar2=None, op0=mybir.AluOpType.add)
                        ob_ps = psp.tile([P, OUT], F32, tag="mm", bufs=4)
                        nc.tensor.transpose(out=ob_ps[:], in_=oT[:], identity=ident[:OUT, :OUT])
                        nc.vector.tensor_copy(out=stage[:, b, :OUT], in_=ob_ps[:])
                if li < L:
                    nc.sync.dma_start(out=shards[li][:].rearrange("(b p) d -> p b d", p=P), in_=stage[:])
                    nc.gpsimd.collective_compute(
                        "AllGather", mybir.AluOpType.bypass, replica_groups=RG,
                        ins=[shards[li][:]], outs=[tables[li][:]],
                    )
                else:
                    nc.sync.dma_start(out=out_ext[:].rearrange("(b p) d -> p b d", p=P), in_=stage[:, :, :OUT])
    return nc


def _run(cfg, inputs):
    in_maps, meta = host_prep(
        inputs["x"], inputs["edge_index"], inputs["W0"], inputs["b0"], inputs["Ws"],
        inputs["bn_gamma"], inputs["bn_beta"], inputs["W_out"], inputs["b_out"], cfg,
    )
    nc = build_with_meta(cfg, meta)
    res = run_bass_kernel_spmd(nc, in_maps, list(range(CORES)))
    N, OUT = cfg["N"], cfg["OUT"]
    SLOTS, order = meta["SLOTS"], meta["order"]
    out = np.zeros((N, OUT), dtype=np.float32)
    for c in range(CORES):
        oc = res.results[c]["out"]  # [SLOTS, OUT]
        r = np.arange(SLOTS) * CORES + c
        valid = r < N
        out[order[r[valid]]] = oc[valid]
    return out


def build_with_meta(cfg, meta):
    return build(cfg, meta)


def kernel(**inputs):
    return _run(FULL, inputs)



# revision 6
# speedup vs baseline: 2.2524x; 2.2524x over previous
"""GCN2 (GCNII) on 8 Trainium2 NeuronCores.

Strategy: nodes degree-sorted; per 8-rank group the 8 nodes are split 4/4
between cores 0-3 (table half 0) and cores 4-7 (half 1), with a host greedy
that balances each target's in-neighbor count across halves (minimizes ELL
padding).  The h~ = dinv*h table is fp16 with rows padded to 128 features
(256B) and exchanged per layer with an AllGather.  Aggregation is bulk
dma_gather (<=1024 idx/call, int16 indices relative to the 25088-row half)
into chunked SBUF staging, reduced per (block, half) with transposed-view
DVE tensor_reduce.  Epilogue per block: z = 0.9*dinv*acc + 0.1*x0, PE
transpose + matmul with ((1-b)I + bW), BatchNorm stats via ACT accum +
AllReduce, relu-apply, dinv-scale back into the fp16 staging shard.
"""
import numpy as np

import bass_rust
from concourse import bass, mybir
from concourse.bass_utils import run_bass_kernel_spmd
from concourse import tile as _tile
from concourse.library_config import mlp as _mlp_lib

# ---------------------------------------------------------------- tctx patch
MAX_WAITS_PER_INST = 1


def _split_sync_waits(nc, max_waits=MAX_WAITS_PER_INST):
    for bb in nc.main_func.blocks:
        out = []
        changed = False
        for ins in bb.instructions:
            si = ins.sync_info
            waits = list(si.on_wait) if si is not None else []
            if len(waits) > max_waits:
                changed = True
                extra, keep = waits[:-max_waits], waits[-max_waits:]
                for i in range(0, len(extra), max_waits):
                    nop = bass_rust.InstNoOp(
                        name=nc.get_next_instruction_name(), text_hint="wsplit"
                    )
                    nop.engine = ins.engine
                    nop.sync_info = mybir.SyncInfo(
                        on_wait=extra[i : i + max_waits], on_update=[]
                    )
                    nc.register_instruction(nop, overwrite=True)
                    out.append(nop)
                ins.sync_info = mybir.SyncInfo(on_wait=keep, on_update=list(si.on_update))
            out.append(ins)
        if changed:
            bb.instructions = out


class TC(_tile.TileContext):
    def __exit__(self, *args):
        r = super().__exit__(*args)
        _split_sync_waits(self.nc)
        return r


# ---------------------------------------------------------------- config
FULL = dict(N=50000, E=800000, IN=500, HID=96, OUT=40, LAYERS=8)
P = 128
CORES = 8
ELEM = 128          # fp16 table row: 96 features + 32 pad = 256B
CALL_IDX = 1024     # SWDGE ring bound per dma_gather call
CHUNK_COLS = 176    # staging columns per chunk (44KB/partition fp16)
ALPHA, THETA, BN_EPS = 0.1, 0.5, 1e-5
F32 = mybir.dt.float32
F16 = mybir.dt.float16
I16 = mybir.dt.int16


def _wrap_call(vals):
    """[n] int64 -> [16, n//16] int16 wrapped (i%16, i//16)."""
    n = len(vals)
    w = np.zeros((16, n // 16), dtype=np.int16)
    i = np.arange(n)
    w[i % 16, i // 16] = vals.astype(np.uint16).view(np.int16)
    return w


def host_prep(x, edge_index, W0, b0, Ws, bn_gamma, bn_beta, W_out, b_out, cfg):
    N, E, IN, HID, OUT, L = (cfg[k] for k in ("N", "E", "IN", "HID", "OUT", "LAYERS"))
    SLOTS = ((N // CORES) + P - 1) // P * P      # 6272
    B = SLOTS // P                               # 49
    NGRP = N // CORES                            # 6250 (exact)
    HTOT = (CORES // 2) * SLOTS                  # half-table rows = 25088
    row, col = np.asarray(edge_index[0]).astype(np.int64), np.asarray(edge_index[1]).astype(np.int64)
    deg = np.bincount(col, minlength=N).astype(np.float32) + 1.0  # + self loop
    dinv = 1.0 / np.sqrt(deg)

    order = np.argsort(-deg, kind="stable")      # rank -> node
    rank = np.empty(N, dtype=np.int64)
    rank[order] = np.arange(N)
    grp = rank // CORES                          # 8-rank group == slot index

    # ---- greedy half assignment: balance each target's in-count across halves
    o = np.argsort(row, kind="stable")
    ts_all = col[o]
    starts = np.searchsorted(row[o], np.arange(N))
    ends = np.searchsorted(row[o], np.arange(N) + 1)
    proc = np.argsort(-(ends - starts), kind="stable")
    cnt = np.zeros((2, N), np.int32)
    gcnt = np.zeros((NGRP, 2), np.int32)
    half = np.zeros(N, np.int8)
    for n in proc:
        t = ts_all[starts[n] : ends[n]]
        g = grp[n]
        if gcnt[g, 0] >= 4:
            h = 1
        elif gcnt[g, 1] >= 4:
            h = 0
        else:
            d0 = int(cnt[0, t].sum()) + int(cnt[0, n])
            d1 = int(cnt[1, t].sum()) + int(cnt[1, n])
            h = 0 if d0 <= d1 else 1
        half[n] = h
        np.add.at(cnt[h], t, 1)
        cnt[h, n] += 1
        gcnt[g, h] += 1

    # ---- node -> (core, slot): within each group, half0 nodes fill cores 0-3,
    # half1 fill 4-7, in rank order.
    node_core = np.empty(N, np.int64)
    node_slot = grp.copy()
    fill = np.zeros((NGRP, 2), np.int64)
    for r in range(N):                            # ranks in order
        n = order[r]
        g = grp[n]
        h = half[n]
        node_core[n] = (0 if h == 0 else 4) + fill[g, h]
        fill[g, h] += 1
    table_row = node_core * SLOTS + node_slot
    rel_row = (table_row - (node_core >= 4) * HTOT).astype(np.int64)  # < 25088
    ZERO_REL = SLOTS - 1                          # slot 6271 of core 0 / core 4: always zero

    nodes_at = np.full((CORES, SLOTS), -1, np.int64)
    nodes_at[node_core, node_slot] = np.arange(N)

    # per-(core,half) ELL arrays over slots
    tgt_core_e = node_core[col]
    tgt_slot_e = node_slot[col]
    src_half_e = half[row]
    src_rel_e = rel_row[row]

    # per-slot counts per half are exactly cnt[h, node]
    D0 = np.zeros(B, np.int64)
    D1 = np.zeros(B, np.int64)
    for b in range(B):
        sl = slice(b * P, (b + 1) * P)
        nb = nodes_at[:, sl]
        valid = nb >= 0
        c0 = np.where(valid, cnt[0, np.maximum(nb, 0)], 0)
        c1 = np.where(valid, cnt[1, np.maximum(nb, 0)], 0)
        D0[b] = int(c0.max())
        D1[b] = int(c1.max())

    DMAX = int(max(D0.max(), D1.max()))

    ell = np.full((CORES, 2, SLOTS, DMAX), ZERO_REL, dtype=np.int64)
    pos = np.zeros((CORES, 2, SLOTS), np.int64)
    # self entries first
    for c in range(CORES):
        nb = nodes_at[c, :NGRP]
        hs = half[nb]
        ell[c, hs, np.arange(NGRP), 0] = rel_row[nb]
        pos[c, hs, np.arange(NGRP)] = 1
    # edges grouped by (core, half, slot)
    for c in range(CORES):
        for h in (0, 1):
            m = (tgt_core_e == c) & (src_half_e == h)
            s_e = tgt_slot_e[m]
            r_e = src_rel_e[m]
            oo = np.argsort(s_e, kind="stable")
            s_s, r_s = s_e[oo], r_e[oo]
            within = np.arange(len(s_s)) - np.searchsorted(s_s, s_s)
            p0 = pos[c, h, s_s]
            ell[c, h, s_s, p0 + within] = r_s

    # ---- chunks of blocks
    chunks = []
    cur, cols = [], 0
    for b in range(B):
        w = int(D0[b] + D1[b])
        if cur and cols + w > CHUNK_COLS:
            chunks.append(cur)
            cur, cols = [], 0
        cur.append(b)
        cols += w
    if cur:
        chunks.append(cur)

    # ---- column layout + call list (shared across cores), idx streams per core
    # chunk layout: [h0 segments of its blocks][h1 segments]
    seg = {}           # (b, h) -> col offset within chunk
    calls = []         # (chunk_id, half, dst_col_off, n_idx, idx_col_off)
    idx_streams = [[] for _ in range(CORES)]
    idx_col = 0
    chunk_meta = []
    for ci, blks in enumerate(chunks):
        off = 0
        for h in (0, 1):
            h_start = off
            streams = [[] for _ in range(CORES)]
            for b in blks:
                D = int(D0[b] if h == 0 else D1[b])
                seg[(b, h)] = off
                if D == 0:
                    continue
                sl = slice(b * P, (b + 1) * P)
                for c in range(CORES):
                    # [P, D] -> column-major stream (k, then p)
                    streams[c].append(ell[c, h, sl, :D].T.ravel())
                off += D
            h_cols = off - h_start
            if h_cols == 0:
                continue
            full = [np.concatenate(s) for s in streams]
            n_total = h_cols * P
            done = 0
            while done < n_total:
                n_i = min(CALL_IDX, n_total - done)
                for c in range(CORES):
                    idx_streams[c].append(_wrap_call(full[c][done : done + n_i]))
                calls.append((ci, h, h_start + done // P, n_i, idx_col))
                idx_col += n_i // 16
                done += n_i
        chunk_meta.append((blks, off))

    IDXCOLS = idx_col
    idx_maps = []
    for c in range(CORES):
        m = np.concatenate(idx_streams[c], axis=1)
        assert m.shape == (16, IDXCOLS)
        idx_maps.append(np.tile(m, (P // 16, 1)))

    # ---- per-core dense inputs
    x_shards, d9s, d10s, d1s, masks = [], [], [], [], []
    x_np = np.asarray(x)
    for c in range(CORES):
        nb = nodes_at[c]
        valid = nb >= 0
        nd = np.maximum(nb, 0)
        xs = np.zeros((SLOTS, IN), dtype=np.float32)
        xs[valid] = x_np[nd[valid]]
        x_shards.append(xs)
        dv = np.where(valid, dinv[nd], 0.0).astype(np.float32)
        d9s.append((0.9 * dv).reshape(B, P).T.copy())
        d1s.append(dv.reshape(B, P).T.copy())
        d10s.append((10.0 * dv).reshape(B, P).T.copy())
        masks.append(valid.astype(np.float32).reshape(B, P).T.copy())

    Ms = np.zeros((L * HID, HID), dtype=np.float32)
    for i in range(1, L + 1):
        bt = float(np.log(THETA / i + 1.0))
        Ms[(i - 1) * HID : i * HID] = (1 - bt) * np.eye(HID, dtype=np.float32) + bt * np.asarray(Ws)[i - 1]
    bnT = np.concatenate([np.asarray(bn_gamma).T, np.asarray(bn_beta).T], axis=1).astype(np.float32)

    meta = dict(
        SLOTS=SLOTS, B=B, HTOT=HTOT, NGRP=NGRP, IDXCOLS=IDXCOLS,
        chunks=chunk_meta, calls=calls, seg=seg, D0=D0, D1=D1,
        node_core=node_core, node_slot=node_slot,
    )
    shared = dict(
        W0=np.asarray(W0, np.float32), Ms=Ms, bnT=bnT,
        W_out=np.asarray(W_out, np.float32),
        b_out=np.asarray(b_out, np.float32).reshape(OUT, 1),
        ident=np.eye(P, dtype=np.float32),
    )
    in_maps = []
    for c in range(CORES):
        m = dict(shared)
        m.update(x=x_shards[c], idx=idx_maps[c], d9=d9s[c], d10=d10s[c],
                 d1=d1s[c], mask=masks[c])
        in_maps.append(m)
    return in_maps, meta


def build(cfg, meta):
    N, IN, HID, OUT, L = (cfg[k] for k in ("N", "IN", "HID", "OUT", "LAYERS"))
    SLOTS, B, HTOT, IDXCOLS = meta["SLOTS"], meta["B"], meta["HTOT"], meta["IDXCOLS"]
    chunks, calls, seg = meta["chunks"], meta["calls"], meta["seg"]
    D0, D1 = meta["D0"], meta["D1"]
    TOT = CORES * SLOTS
    KC = (IN + P - 1) // P
    ksz = [min(P, IN - i * P) for i in range(KC)]

    nc = bass.Bass()
    x_in = nc.declare_dram_parameter("x", [SLOTS, IN], F32, isOutput=False)
    idx_in = nc.declare_dram_parameter("idx", [P, IDXCOLS], I16, isOutput=False)
    d9_in = nc.declare_dram_parameter("d9", [P, B], F32, isOutput=False)
    d10_in = nc.declare_dram_parameter("d10", [P, B], F32, isOutput=False)
    d1_in = nc.declare_dram_parameter("d1", [P, B], F32, isOutput=False)
    mask_in = nc.declare_dram_parameter("mask", [P, B], F32, isOutput=False)
    W0_in = nc.declare_dram_parameter("W0", [IN, HID], F32, isOutput=False)
    Ms_in = nc.declare_dram_parameter("Ms", [L * HID, HID], F32, isOutput=False)
    bnT_in = nc.declare_dram_parameter("bnT", [HID, 2 * (L + 1)], F32, isOutput=False)
    Wout_in = nc.declare_dram_parameter("W_out", [HID, OUT], F32, isOutput=False)
    bout_in = nc.declare_dram_parameter("b_out", [OUT, 1], F32, isOutput=False)
    id_in = nc.declare_dram_parameter("ident", [P, P], F32, isOutput=False)
    out_ext = nc.declare_dram_parameter("out", [SLOTS, OUT], F32, isOutput=True)

    shards = [nc.dram_tensor(f"shard{i}", [SLOTS, ELEM], F16) for i in range(L)]
    tables = [nc.dram_tensor(f"table{i}", [TOT, ELEM], F16) for i in range(L)]
    st_in = [nc.dram_tensor(f"stin{i}", [2 * HID, 1], F32) for i in range(L + 1)]
    st_out = [nc.dram_tensor(f"stout{i}", [2 * HID, 1], F32, addr_space="Shared") for i in range(L + 1)]
    RG = [list(range(CORES))]

    with TC(nc, num_cores=CORES) as tc:
        with (
            tc.tile_pool(name="persist", bufs=1) as pp,
            tc.tile_pool(name="work", bufs=3) as wp,
            tc.tile_pool(name="stgp", bufs=2) as sp,
            tc.tile_pool(name="psum", bufs=1, space="PSUM") as psp,
        ):
            nc.gpsimd.load_library(_mlp_lib)
            idx_t = pp.tile([P, IDXCOLS], I16, tag="idx")
            nc.sync.dma_start(out=idx_t[:], in_=idx_in[:])
            d9_t = pp.tile([P, B], F32, tag="d9")
            nc.sync.dma_start(out=d9_t[:], in_=d9_in[:])
            d10_t = pp.tile([P, B], F32, tag="d10")
            nc.sync.dma_start(out=d10_t[:], in_=d10_in[:])
            d1_t = pp.tile([P, B], F32, tag="d1")
            nc.sync.dma_start(out=d1_t[:], in_=d1_in[:])
            mask_t = pp.tile([P, B], F32, tag="mask")
            nc.sync.dma_start(out=mask_t[:], in_=mask_in[:])
            ident = pp.tile([P, P], F32, tag="id")
            nc.sync.dma_start(out=ident[:], in_=id_in[:])
            W0c = []
            for kc in range(KC):
                kw = ksz[kc]
                w = pp.tile([P, HID], F32, tag=f"w0_{kc}")
                nc.sync.dma_start(out=w[:kw, :], in_=W0_in[kc * P : kc * P + kw, :])
                W0c.append(w)
            Wo_t = pp.tile([HID, OUT], F32, tag="wo")
            nc.sync.dma_start(out=Wo_t[:], in_=Wout_in[:])
            bo_t = pp.tile([OUT, 1], F32, tag="bo")
            nc.sync.dma_start(out=bo_t[:], in_=bout_in[:])
            bnT_t = pp.tile([HID, 2 * (L + 1)], F32, tag="bn")
            nc.sync.dma_start(out=bnT_t[:], in_=bnT_in[:])


            x0s = pp.tile([P, B, HID], F32, tag="x0s")       # 0.1 * x0
            stage = pp.tile([P, B, ELEM], F16, tag="stage")  # h~ node-major fp16, padded
            nc.vector.memset(stage[:], 0.0)                  # pad cols stay zero
            z1Tst = pp.tile([HID, B, P], F16, tag="z1st")
            z2st = pp.tile([HID, B, P], F16, tag="z2st")
            onm = pp.tile([P, B, OUT], F32, tag="onm")
            stc1 = pp.tile([HID, B], F32, tag="stc1")
            stc2 = pp.tile([HID, B], F32, tag="stc2")

            def bn_vectors(sum_ps, sum2_ps, gcol, bcol, sc, bi):
                m = wp.tile([HID, 1], F32, tag="bnm")
                nc.vector.tensor_scalar_mul(m[:], sum_ps, 1.0 / N)
                v = wp.tile([HID, 1], F32, tag="bnv")
                nc.vector.tensor_scalar_mul(v[:], sum2_ps, 1.0 / N)
                m2 = wp.tile([HID, 1], F32, tag="bnm2")
                nc.vector.tensor_tensor(out=m2[:], in0=m[:], in1=m[:], op=mybir.AluOpType.mult)
                nc.vector.tensor_tensor(out=v[:], in0=v[:], in1=m2[:], op=mybir.AluOpType.subtract)
                nc.vector.tensor_scalar_add(v[:], v[:], BN_EPS)
                sd = wp.tile([HID, 1], F32, tag="bnsd")
                nc.scalar.activation(out=sd[:], in_=v[:], func=mybir.ActivationFunctionType.Sqrt)
                nc.vector.reciprocal(out=sd[:], in_=sd[:])
                nc.vector.tensor_tensor(out=sc[:], in0=bnT_t[:, gcol : gcol + 1], in1=sd[:], op=mybir.AluOpType.mult)
                t = wp.tile([HID, 1], F32, tag="bnt")
                nc.vector.tensor_tensor(out=t[:], in0=m[:], in1=sc[:], op=mybir.AluOpType.mult)
                nc.vector.tensor_tensor(out=bi[:], in0=bnT_t[:, bcol : bcol + 1], in1=t[:], op=mybir.AluOpType.subtract)

            # ---------------- layer 0: z1 = x @ W0, BN0, relu
            for b in range(B):
                xb = wp.tile([P, IN], F32, tag="xb")
                nc.sync.dma_start(out=xb[:], in_=x_in[b * P : (b + 1) * P, :])
                z1_ps = psp.tile([P, HID], F32, tag="z1ps")
                for kc in range(KC):
                    kw = ksz[kc]
                    xt_ps = psp.tile([P, P], F32, tag="mm", bufs=4)
                    nc.tensor.transpose(out=xt_ps[:kw, :], in_=xb[:, kc * P : kc * P + kw], identity=ident[:])
                    xt_sb = wp.tile([P, P], F32, tag="xtsb")
                    nc.vector.tensor_copy(out=xt_sb[:kw, :], in_=xt_ps[:kw, :])
                    nc.tensor.matmul(
                        out=z1_ps[:], lhsT=xt_sb[:kw, :], rhs=W0c[kc][:kw, :],
                        start=(kc == 0), stop=(kc == KC - 1),
                    )
                z1b = wp.tile([P, HID], F32, tag="z1b")
                nc.vector.tensor_copy(out=z1b[:], in_=z1_ps[:])
                z1T_ps = psp.tile([HID, P], F32, tag="mm", bufs=4)
                nc.tensor.transpose(out=z1T_ps[:], in_=z1b[:], identity=ident[:])
                nc.scalar.activation(out=z1Tst[:, b, :], in_=z1T_ps[:], func=mybir.ActivationFunctionType.Copy, accum_out=stc1[:, b : b + 1])
                sq2 = wp.tile([HID, P], F16, tag="sq2")
                nc.scalar.activation(out=sq2[:], in_=z1T_ps[:], func=mybir.ActivationFunctionType.Square, accum_out=stc2[:, b : b + 1])
            rs1 = wp.tile([HID, 1], F32, tag="rs1")
            nc.vector.tensor_reduce(out=rs1[:], in_=stc1[:], axis=mybir.AxisListType.X, op=mybir.AluOpType.add)
            rs2 = wp.tile([HID, 1], F32, tag="rs2")
            nc.vector.tensor_reduce(out=rs2[:], in_=stc2[:], axis=mybir.AxisListType.X, op=mybir.AluOpType.add)
            nc.sync.dma_start(out=st_in[0][0:HID, :], in_=rs1[:])
            nc.sync.dma_start(out=st_in[0][HID : 2 * HID, :], in_=rs2[:])
            nc.gpsimd.collective_compute(
                "AllReduce", mybir.AluOpType.add, replica_groups=RG,
                ins=[st_in[0][:]], outs=[st_out[0][:]],
            )
            asum1 = wp.tile([HID, 1], F32, tag="as1")
            nc.sync.dma_start(out=asum1[:], in_=st_out[0][0:HID, :])
            asum2 = wp.tile([HID, 1], F32, tag="as2")
            nc.sync.dma_start(out=asum2[:], in_=st_out[0][HID : 2 * HID, :])
            sc0 = pp.tile([HID, 1], F32, tag="sc0")
            bi0 = pp.tile([HID, 1], F32, tag="bi0")
            bn_vectors(asum1[:], asum2[:], 0, L + 1, sc0, bi0)
            nc.vector.tensor_scalar_mul(sc0[:], sc0[:], 0.1)
            nc.vector.tensor_scalar_mul(bi0[:], bi0[:], 0.1)
            for b in range(B):
                x0T = wp.tile([HID, P], F32, tag="x0T")
                nc.scalar.activation(out=x0T[:], in_=z1Tst[:, b, :], func=mybir.ActivationFunctionType.Relu, scale=sc0[:], bias=bi0[:])
                x0b_ps = psp.tile([P, HID], F32, tag="mm", bufs=4)
                nc.tensor.transpose(out=x0b_ps[:], in_=x0T[:], identity=ident[:HID, :HID])
                nc.vector.tensor_scalar(out=x0s[:, b, :], in0=x0b_ps[:], scalar1=mask_t[:, b : b + 1], scalar2=None, op0=mybir.AluOpType.mult)
                nc.vector.tensor_scalar(out=stage[:, b, 0:HID], in0=x0s[:, b, :], scalar1=d10_t[:, b : b + 1], scalar2=None, op0=mybir.AluOpType.mult)
            nc.sync.dma_start(out=shards[0][:].rearrange("(b p) d -> p b d", p=P), in_=stage[:])
            nc.gpsimd.collective_compute(
                "AllGather", mybir.AluOpType.bypass, replica_groups=RG,
                ins=[shards[0][:]], outs=[tables[0][:]],
            )

            # ---------------- layers 1..L
            _reg_cache = {}

            def nreg(v):
                if v not in _reg_cache:
                    _reg_cache[v] = nc.gpsimd.to_reg(v)
                return _reg_cache[v]

            for li in range(1, L + 1):
                tbl = tables[li - 1]
                Ms_t_cur = wp.tile([HID, HID], F32, tag="ms")
                nc.sync.dma_start(out=Ms_t_cur[:], in_=Ms_in[(li - 1) * HID : li * HID, :])
                skip_bn = li == L - 1

                chunk_tiles = {}
                for ci, (blks, used_cols) in enumerate(chunks):
                    chunk_tiles[ci] = sp.tile([P, CHUNK_COLS, ELEM], F16, tag="stg", name=f"stg{li}_{ci}")
                for (ci, h, dst_off, n_idx, idx_off) in calls:
                    stg = chunk_tiles[ci]
                    ncol = n_idx // P
                    src = tbl[0:HTOT, :] if h == 0 else tbl[HTOT : 2 * HTOT, :]
                    nc.gpsimd.dma_gather(
                        out_ap=stg[:, dst_off : dst_off + ncol, :], in_ap=src,
                        idxs_ap=idx_t[:, idx_off : idx_off + n_idx // 16],
                        num_idxs=n_idx, num_idxs_reg=nreg(n_idx), elem_size=ELEM,
                    )
                for ci, (blks, used_cols) in enumerate(chunks):
                    stg = chunk_tiles[ci]
                    for b in blks:
                        dd0, dd1 = int(D0[b]), int(D1[b])
                        t0 = wp.tile([P, HID], F32, tag="t0")
                        s0 = seg[(b, 0)]
                        if dd0 > 0:
                            nc.vector.tensor_reduce(
                                out=t0[:], in_=stg[:, s0 : s0 + dd0, 0:HID].rearrange("p c e -> p e c"),
                                axis=mybir.AxisListType.X, op=mybir.AluOpType.add,
                            )
                        else:
                            nc.vector.memset(t0[:], 0.0)
                        acc = wp.tile([P, HID], F32, tag="acc")
                        s1c = seg[(b, 1)]
                        if dd1 > 0:
                            nc.vector.tensor_reduce(
                                out=acc[:], in_=stg[:, s1c : s1c + dd1, 0:HID].rearrange("p c e -> p e c"),
                                axis=mybir.AxisListType.X, op=mybir.AluOpType.add,
                            )
                        else:
                            nc.vector.memset(acc[:], 0.0)
                        nc.vector.tensor_tensor(out=acc[:], in0=acc[:], in1=t0[:], op=mybir.AluOpType.add)
                        zb = wp.tile([P, HID], F32, tag="zb")
                        nc.vector.scalar_tensor_tensor(
                            out=zb[:], in0=acc[:], scalar=d9_t[:, b : b + 1],
                            in1=x0s[:, b, :], op0=mybir.AluOpType.mult, op1=mybir.AluOpType.add,
                        )
                        zT_ps = psp.tile([HID, P], F32, tag="mm", bufs=4)
                        nc.tensor.transpose(out=zT_ps[:], in_=zb[:], identity=ident[:])
                        zT = wp.tile([HID, P], F32, tag="zT")
                        nc.vector.tensor_copy(out=zT[:], in_=zT_ps[:])
                        z2_ps = psp.tile([HID, P], F32, tag="mm", bufs=4)
                        nc.tensor.matmul(out=z2_ps[:], lhsT=Ms_t_cur[:], rhs=zT[:], start=True, stop=True)
                        if skip_bn:
                            nc.vector.tensor_copy(out=z2st[:, b, :], in_=z2_ps[:])
                        else:
                            nc.scalar.activation(out=z2st[:, b, :], in_=z2_ps[:], func=mybir.ActivationFunctionType.Copy, accum_out=stc1[:, b : b + 1])
                            sq2 = wp.tile([HID, P], F32, tag="sq2")
                            nc.scalar.activation(out=sq2[:], in_=z2st[:, b, :], func=mybir.ActivationFunctionType.Square, accum_out=stc2[:, b : b + 1])
                if not skip_bn:
                    rs1 = wp.tile([HID, 1], F32, tag="rs1")
                    nc.vector.tensor_reduce(out=rs1[:], in_=stc1[:], axis=mybir.AxisListType.X, op=mybir.AluOpType.add)
                    rs2 = wp.tile([HID, 1], F32, tag="rs2")
                    nc.vector.tensor_reduce(out=rs2[:], in_=stc2[:], axis=mybir.AxisListType.X, op=mybir.AluOpType.add)
                    nc.sync.dma_start(out=st_in[li][0:HID, :], in_=rs1[:])
                    nc.sync.dma_start(out=st_in[li][HID : 2 * HID, :], in_=rs2[:])
                    nc.gpsimd.collective_compute(
                        "AllReduce", mybir.AluOpType.add, replica_groups=RG,
                        ins=[st_in[li][:]], outs=[st_out[li][:]],
                    )
                    g1 = wp.tile([HID, 1], F32, tag="g1")
                    nc.sync.dma_start(out=g1[:], in_=st_out[li][0:HID, :])
                    g2 = wp.tile([HID, 1], F32, tag="g2")
                    nc.sync.dma_start(out=g2[:], in_=st_out[li][HID : 2 * HID, :])
                    sc = wp.tile([HID, 1], F32, tag="sc")
                    bi = wp.tile([HID, 1], F32, tag="bi")
                    bn_vectors(g1[:], g2[:], li, (L + 1) + li, sc, bi)
                for b in range(B):
                    hT = wp.tile([HID, P], F32, tag="hT")
                    if skip_bn:
                        nc.scalar.activation(out=hT[:], in_=z2st[:, b, :], func=mybir.ActivationFunctionType.Relu)
                    else:
                        nc.scalar.activation(out=hT[:], in_=z2st[:, b, :], func=mybir.ActivationFunctionType.Relu, scale=sc[:], bias=bi[:])
                    if li < L:
                        hb_ps = psp.tile([P, HID], F32, tag="mm", bufs=4)
                        nc.tensor.transpose(out=hb_ps[:], in_=hT[:], identity=ident[:HID, :HID])
                        nc.vector.tensor_scalar(out=stage[:, b, 0:HID], in0=hb_ps[:], scalar1=d1_t[:, b : b + 1], scalar2=None, op0=mybir.AluOpType.mult)
                    else:
                        o_ps = psp.tile([OUT, P], F32, tag="mm", bufs=4)
                        nc.tensor.matmul(out=o_ps[:], lhsT=Wo_t[:], rhs=hT[:], start=True, stop=True)
                        oT = wp.tile([OUT, P], F32, tag="oT")
                        nc.vector.tensor_scalar(out=oT[:], in0=o_ps[:], scalar1=bo_t[:], scalar2=None, op0=mybir.AluOpType.add)
                        ob_ps = psp.tile([P, OUT], F32, tag="mm", bufs=4)
                        nc.tensor.transpose(out=ob_ps[:], in_=oT[:], identity=ident[:OUT, :OUT])
                        nc.vector.tensor_copy(out=onm[:, b, :], in_=ob_ps[:])
                if li < L:
                    nc.sync.dma_start(out=shards[li][:].rearrange("(b p) d -> p b d", p=P), in_=stage[:])
                    nc.gpsimd.collective_compute(
                        "AllGather", mybir.AluOpType.bypass, replica_groups=RG,
                        ins=[shards[li][:]], outs=[tables[li][:]],
                    )
                else:
                    nc.sync.dma_start(out=out_ext[:].rearrange("(b p) d -> p b d", p=P), in_=onm[:])
    mybir.codegen_inst_isa_subclasses(nc)
    return nc


def _run(cfg, inputs):
    in_maps, meta = host_prep(
        inputs["x"], inputs["edge_index"], inputs["W0"], inputs["b0"], inputs["Ws"],
        inputs["bn_gamma"], inputs["bn_beta"], inputs["W_out"], inputs["b_out"], cfg,
    )
    nc = build_with_meta(cfg, meta)
    res = run_bass_kernel_spmd(nc, in_maps, list(range(CORES)))
    N, OUT = cfg["N"], cfg["OUT"]
    SLOTS = meta["SLOTS"]
    node_core, node_slot = meta["node_core"], meta["node_slot"]
    out = np.zeros((N, OUT), dtype=np.float32)
    for c in range(CORES):
        oc = res.results[c]["out"]  # [SLOTS, OUT]
        m = node_core == c
        out[m] = oc[node_slot[m]]
    return out


def build_with_meta(cfg, meta):
    return build(cfg, meta)


def kernel(**inputs):
    return _run(FULL, inputs)
